# revision 44
# baseline (speedup 1.0000x reference)
"""Trainium2 Bass kernel: 2D dense-grid embedding lookup (bilinear interpolation).

Problem (hardcoded shapes):
  inputs:     [65536, 2]  fp32 uniform [0,1)
  embeddings: [16384, 1024] fp32  (128x128 grid, D=1024 features)
  out[b, :] = sum_c w_c(b) * embeddings[id_c(b), :]   (4 bilinear corners)

Strategy (grid-band sharding + one-hot matmul "gather" on the PE):
  - Indirect DRAM gathers (the naive approach) re-read 8-16KB per point and
    are descriptor-rate limited (~470us). Instead each point's 4-corner blend
    is computed as a one-hot matmul: out[128 pts, 1024] = W^T @ E_slot with
    W [K=128, M=128 pts] fp16 holding the bilinear weights, accumulated in
    fp32 PSUM. Measured: no HBM contention across the 8 cores (1-core ==
    8-core per-rep time), so the design minimizes per-engine work, PE first.
  - K-packing: one K=128 covers BOTH bilinear grid rows x a 64-column window
    (k = 64*band + (col - c0)), so each chunk of <=128 points needs only two
    N=512 matmuls. The input points cluster in narrow column windows per
    grid row, so the host greedily packs each grid row's points (sorted by
    column) into chunks whose corner-column span fits a 64-wide window.
  - The rhs table slices E2[:, slot, :] = table rows {g,g+1} x cols
    [c0, c0+64) are built BY THE HOST per (core, slot): all row/window
    metadata lives in tensor content, so the NEFF is identical across cores
    (SPMD) and fully static: chunk ch always reads slot ch//2 (chunks are
    paired per slot; odd leftovers get a zero-weight pad chunk).
  - Per-core sharding by grid-row band (xi0 // 16, ~8200 pts each).
    PSUM -> SBUF fp16 copies alternate DVE/ACT (GPSIMD cannot touch PSUM);
    stores are contiguous 256KB DMAs and, with the loads, rotate over all
    three DMA queues (SP/ACT HWDGE + Pool SWDGE) so matmuls overlap DMA.
  - Host inverse-permutes core outputs to original point order, fp32 upcast
    (rel err ~1e-3 from fp16 table/weights/output, under the 2e-2 gate).
"""

import numpy as np

RES = 128
B_TOTAL = 65536
N_CORES = 8
D = 1024
P = 128
G_PER_CORE = RES // N_CORES  # 16 grid rows per core
SPAN = 62  # max (col - chunk c0) so col+1 corners stay in the 64-col window

_CACHED = {}  # (nslot,) -> nc
_LAST_PREP = None


def _emit(tc, w_ap, e2_ap, out_ap, nslot, repeat=1):
    from concourse import mybir

    nc = tc.nc
    f16 = mybir.dt.float16
    f32 = mybir.dt.float32
    nch = 2 * nslot

    from contextlib import ExitStack

    ctx = ExitStack()
    persist = ctx.enter_context(tc.tile_pool(name="persist", bufs=1))
    opool = ctx.enter_context(tc.tile_pool(name="out", bufs=8))
    ppool = ctx.enter_context(tc.tile_pool(name="psum", bufs=4, space="PSUM"))

    E_sb = persist.tile([P, nslot * D], f16, tag="E", name="E")
    W_sb = persist.tile([P, nch * P], f16, tag="W", name="W")

    def copy_dve(o, i):
        nc.vector.tensor_copy(o, i)

    def copy_act(o, i):
        nc.scalar.copy(o, i)

    # GPSIMD/Pool cannot access PSUM (BIR verifier), so copies are DVE/ACT only
    copy_engs = (copy_dve, copy_act)
    load_engs = (nc.sync, nc.scalar, nc.gpsimd)
    store_engs = (nc.sync, nc.scalar, nc.gpsimd)

    def body():
        # E2/W loads sliced per slot, spread across the three DMA-issue
        # engines so chunk 0's matmuls start before the tail of the load.
        for s in range(nslot):
            load_engs[s % 3].dma_start(
                out=E_sb[:, s * D : (s + 1) * D],
                in_=e2_ap[:, s * D : (s + 1) * D],
            )
            lo, hi = s * 2 * P, (s + 1) * 2 * P
            load_engs[(s + 1) % 3].dma_start(
                out=W_sb[:, lo:hi], in_=w_ap[:, lo:hi]
            )

        for ch in range(nch):
            s = ch // 2
            W_ch = W_sb[:, ch * P : (ch + 1) * P]
            ps = ppool.tile([P, D], f32, tag="ps", name="ps")
            for h in range(2):
                nc.tensor.matmul(
                    ps[:, h * 512 : (h + 1) * 512],
                    lhsT=W_ch,
                    rhs=E_sb[:, s * D + h * 512 : s * D + h * 512 + 512],
                    start=True,
                    stop=True,
                )
            O = opool.tile([P, D], f16, tag="O", name="O")
            copy_engs[ch % 2](O[:], ps[:])
            store_engs[ch % 3].dma_start(
                out=out_ap[ch * P : (ch + 1) * P, :], in_=O[:]
            )

    # repeat>1 re-runs the identical work in a hardware loop (timing-slope
    # measurement only); loads are inside so the slope covers the full kernel.
    if repeat == 1:
        body()
    else:
        with tc.For_i(0, repeat):
            body()

    ctx.close()


def build_nc(nslot=None, repeat=1, **emit_kwargs):
    import concourse.tile as tile
    from concourse import bacc, mybir

    if nslot is None:
        nslot = _LAST_PREP["nslot"] if _LAST_PREP else 40
    nch = 2 * nslot
    nc = bacc.Bacc("TRN2", debug=False)
    w = nc.dram_tensor("w", [P, nch * P], mybir.dt.float16, kind="ExternalInput")
    e2 = nc.dram_tensor("e2", [P, nslot * D], mybir.dt.float16,
                        kind="ExternalInput")
    out = nc.dram_tensor("out", [nch * P, D], mybir.dt.float16,
                         kind="ExternalOutput")
    with tile.TileContext(nc) as tc:
        _emit(tc, w[:], e2[:], out[:], nslot, repeat=repeat, **emit_kwargs)
    if not nc.is_finalized():
        nc.finalize()
    return nc


def _get_nc(nslot):
    if nslot not in _CACHED:
        _CACHED[nslot] = build_nc(nslot)
    return _CACHED[nslot]


def _host_prep(inputs, embeddings):
    """Greedy-pack points into (row, 64-col-window) chunks; build W and E2."""
    inputs = np.ascontiguousarray(np.asarray(inputs), dtype=np.float32)
    embeddings = np.asarray(embeddings)
    x = inputs * np.float32(RES - 1)
    xi = np.floor(x).astype(np.int32)
    xf = x - np.floor(x)
    xi0, xi1 = xi[:, 0], xi[:, 1]
    xf0, xf1 = xf[:, 0].astype(np.float32), xf[:, 1].astype(np.float32)
    wgt = np.stack(
        [
            (1 - xf0) * (1 - xf1),  # (row g  , col c)
            (1 - xf0) * xf1,        # (row g  , col c+1)
            xf0 * (1 - xf1),        # (row g+1, col c)
            xf0 * xf1,              # (row g+1, col c+1)
        ],
        axis=1,
    )

    order = np.argsort(xi0.astype(np.int64) * RES + xi1, kind="stable")
    srow, scol = xi0[order], xi1[order]

    # Greedy chunking per grid row: chunks of <=128 col-sorted points whose
    # corner-col span fits a 64-col window starting at the chunk's first col.
    # Then pair chunks into slots (slot = ch//2); both chunks of a slot must
    # share the (g, c0) window, else the odd chunk gets an empty pad partner.
    row_of = {}
    for g in range(RES):
        lo, hi = np.searchsorted(srow, [g, g + 1])
        if hi > lo:
            row_of[g] = (lo, hi)
    core_slots = [[] for _ in range(N_CORES)]  # (g, c0, [ptsA, ptsB])
    empty = np.empty(0, dtype=np.int64)
    for g, (lo, hi) in row_of.items():
        cols = scol[lo:hi]
        chunks = []
        i = 0
        while i < hi - lo:
            c0 = int(cols[i])
            j = min(i + P, int(np.searchsorted(cols, c0 + SPAN, "right")))
            chunks.append((c0, order[lo + i : lo + j]))
            i = j
        slots = core_slots[g // G_PER_CORE]
        k = 0
        while k < len(chunks):
            c0, pts = chunks[k]
            if (
                k + 1 < len(chunks)
                and int(xi1[chunks[k + 1][1]].max()) - c0 <= SPAN
            ):
                slots.append((g, c0, [pts, chunks[k + 1][1]]))
                k += 2
            else:
                slots.append((g, c0, [pts, empty]))
                k += 1

    nslot = max(len(s) for s in core_slots)
    nch = 2 * nslot

    emb16 = embeddings.astype(np.float16)
    p_lo = np.arange(64)
    in_maps = []
    orig = np.full((N_CORES, nch * P), -1, dtype=np.int64)
    for c in range(N_CORES):
        W = np.zeros((nch, P, P), dtype=np.float16)  # [ch, k, m]
        E2 = np.zeros((P, nslot, D), dtype=np.float16)
        for s, (g, c0, chunk_pair) in enumerate(core_slots[c]):
            cw = np.minimum(c0 + p_lo, P - 1)  # cols past 127 are never hit
            E2[:64, s, :] = emb16[g * P + cw]
            E2[64:, s, :] = emb16[(g + 1) * P + cw]
            for half, pts in enumerate(chunk_pair):
                if len(pts) == 0:
                    continue
                ch = 2 * s + half
                m = np.arange(len(pts))
                k0 = xi1[pts] - c0
                W[ch, k0, m] = wgt[pts, 0]
                W[ch, k0 + 1, m] = wgt[pts, 1]
                W[ch, 64 + k0, m] = wgt[pts, 2]
                W[ch, 65 + k0, m] = wgt[pts, 3]
                orig[c, ch * P + m] = pts
        in_maps.append(
            {
                "w": np.ascontiguousarray(
                    W.transpose(1, 0, 2).reshape(P, nch * P)
                ),
                "e2": np.ascontiguousarray(E2.reshape(P, nslot * D)),
            }
        )
    return {"in_maps": in_maps, "orig": orig, "nslot": nslot, "nch": nch}


def make_core_inputs(inputs: np.ndarray, embeddings: np.ndarray) -> list:
    global _LAST_PREP
    _LAST_PREP = _host_prep(inputs, embeddings)
    return _LAST_PREP["in_maps"]


def core_output_global(out_core: np.ndarray, core: int):
    """Map one core's raw device output to (global_indices, fp32 values)."""
    prep = _LAST_PREP
    rows = out_core.reshape(prep["nch"] * P, D)
    orig = prep["orig"][core]
    mask = orig >= 0
    return orig[mask], rows[mask].astype(np.float32)


def kernel(inputs: np.ndarray, embeddings: np.ndarray) -> np.ndarray:
    from concourse.bass_utils import run_bass_kernel_spmd

    in_maps = make_core_inputs(inputs, embeddings)
    prep = _LAST_PREP
    nc = _get_nc(prep["nslot"])
    res = run_bass_kernel_spmd(nc, in_maps, core_ids=list(range(N_CORES)))
    out = np.empty((B_TOTAL, D), dtype=np.float32)
    covered = 0
    for c in range(N_CORES):
        gidx, vals = core_output_global(res.results[c]["out"], c)
        out[gidx] = vals
        covered += len(gidx)
    assert covered == B_TOTAL, f"only {covered} of {B_TOTAL} points covered"
    return out


if __name__ == "__main__":
    nc = build_nc()
    print("built ok")


# revision 45
# speedup vs baseline: 1.2340x; 1.2340x over previous
"""Trainium2 Bass kernel: 2D dense-grid embedding lookup (bilinear interpolation).

Problem (hardcoded shapes):
  inputs:     [65536, 2]  fp32 uniform [0,1)
  embeddings: [16384, 1024] fp32  (128x128 grid, D=1024 features)
  out[b, :] = sum_c w_c(b) * embeddings[id_c(b), :]   (4 bilinear corners)

Strategy (grid-row-band sharding + one-hot matmul "gather" on the PE):
  - The table has 16x average reuse (65536 pts x 4 corners over 16384 rows).
    Indirect DRAM gathers (previous approach) re-read 8-16KB per point and are
    descriptor-rate limited. Instead: shard the GRID into 8 bands of 16 grid
    rows; each core keeps its band (17 rows x 128 cols x 1024 feats, fp16,
    4.25MB) resident in SBUF and reads it from HBM exactly once.
  - Host routes each point to the core owning its grid row (xi0 // 16), sorts
    by local grid row, packs points into chunks of 128 with a static
    chunk -> grid-row map (CAP chunks per grid row, zero-padded). For each
    chunk the host builds two one-hot-weighted matrices W_lo/W_hi
    [K=128 grid cols, M=128 points] (fp16) holding the bilinear corner
    weights at rows xi1 and xi1+1.
  - Device per chunk: out[128 pts, 1024] = W_lo^T @ E[g] + W_hi^T @ E[g+1]
    as fp16 matmuls accumulating in fp32 PSUM (1 cycle/row, N=512 per inst).
    PSUM -> SBUF fp16 copies alternate DVE/Pool/ACT; stores are fully
    contiguous 128KB DMAs. No indirect DMA anywhere.
  - Host inverse-permutes core outputs back to the original point order and
    upcasts to fp32 (rel err ~1.5e-3 from fp16 table/weights/output, well
    under the 2e-2 gate).
"""

import numpy as np

RES = 128
B_TOTAL = 65536
N_CORES = 8
D = 1024
P = 128
G_PER_CORE = RES // N_CORES  # 16 grid rows per core
BAND_ROWS = G_PER_CORE + 1  # 17 (halo row for xi0+1 corners)
DEF_CAP = 5  # chunks of 128 points per grid row (max count 571 for key-0 data)

_CACHED = {}  # cap -> nc
_LAST_PREP = None


def _emit(tc, w_ap, band_ap, out_ap, cap, repeat=1, store_mode="copy"):
    from concourse import mybir
    from contextlib import ExitStack

    nc = tc.nc
    f16 = mybir.dt.float16
    f32 = mybir.dt.float32
    nch = G_PER_CORE * cap  # chunks per core

    ctx = ExitStack()
    persist = ctx.enter_context(tc.tile_pool(name="persist", bufs=1))
    opool = ctx.enter_context(tc.tile_pool(name="out", bufs=8))
    ppool = ctx.enter_context(tc.tile_pool(name="psum", bufs=4, space="PSUM"))

    E_sb = persist.tile([P, BAND_ROWS * D], f16, tag="E", name="E")
    W_sb = persist.tile([P, nch * 2 * P], f16, tag="W", name="W")

    def copy_dve(o, i):
        nc.vector.tensor_copy(o, i)

    def copy_act(o, i):
        nc.scalar.copy(o, i)

    copy_engs = (copy_dve, copy_act)
    load_engs = (nc.sync, nc.scalar, nc.gpsimd)
    store_engs = (nc.sync, nc.scalar, nc.gpsimd)

    def body():
        # Band: DRAM [BAND_ROWS, 128, D] -> SBUF [128 cols, (g d)], sliced
        # per grid row + W per row-group, spread over 3 DMA queues so the
        # first chunks' matmuls can start before the tail of the load.
        for g in range(BAND_ROWS):
            load_engs[g % 3].dma_start(
                out=E_sb[:, g * D : (g + 1) * D], in_=band_ap[g]
            )
        for g in range(G_PER_CORE):
            lo, hi = g * cap * 2 * P, (g + 1) * cap * 2 * P
            load_engs[(g + BAND_ROWS) % 3].dma_start(
                out=W_sb[:, lo:hi], in_=w_ap[:, lo:hi]
            )

        for ch in range(nch):
            g = ch // cap
            W_lo = W_sb[:, ch * 2 * P : ch * 2 * P + P]
            W_hi = W_sb[:, ch * 2 * P + P : (ch + 1) * 2 * P]
            ps = ppool.tile([P, D], f32, tag="ps", name="ps")
            for h in range(2):
                psh = ps[:, h * 512 : (h + 1) * 512]
                rhs_lo = E_sb[:, g * D + h * 512 : g * D + h * 512 + 512]
                rhs_hi = E_sb[:, (g + 1) * D + h * 512 : (g + 1) * D + h * 512 + 512]
                nc.tensor.matmul(psh, lhsT=W_lo, rhs=rhs_lo, start=True, stop=False)
                nc.tensor.matmul(psh, lhsT=W_hi, rhs=rhs_hi, start=False, stop=True)
            if store_mode == "cast":
                # SWDGE casting store straight from PSUM fp32 -> DRAM fp16
                nc.gpsimd.dma_start(
                    out=out_ap[ch * P : (ch + 1) * P, :], in_=ps[:]
                )
            else:
                O = opool.tile([P, D], f16, tag="O", name="O")
                copy_engs[ch % 2](O[:], ps[:])
                store_engs[ch % 3].dma_start(
                    out=out_ap[ch * P : (ch + 1) * P, :], in_=O[:]
                )

    # repeat>1 re-runs the identical work in a hardware loop (timing-slope
    # measurement only); loads are inside so the slope covers the full kernel.
    if repeat == 1:
        body()
    else:
        with tc.For_i(0, repeat):
            body()

    ctx.close()


def build_nc(cap=None, repeat=1, **emit_kwargs):
    if cap is None:
        cap = _LAST_PREP["cap"] if _LAST_PREP else DEF_CAP
    import concourse.tile as tile
    from concourse import bacc, mybir

    nch = G_PER_CORE * cap
    nc = bacc.Bacc("TRN2", debug=False)
    w = nc.dram_tensor("w", [P, nch * 2 * P], mybir.dt.float16, kind="ExternalInput")
    band = nc.dram_tensor(
        "band", [BAND_ROWS, P, D], mybir.dt.float16, kind="ExternalInput"
    )
    out = nc.dram_tensor(
        "out", [nch * P, D], mybir.dt.float16, kind="ExternalOutput"
    )
    with tile.TileContext(nc) as tc:
        _emit(tc, w[:], band[:], out[:], cap, repeat=repeat, **emit_kwargs)
    if not nc.is_finalized():
        nc.finalize()
    return nc


def _get_nc(cap):
    if cap not in _CACHED:
        _CACHED[cap] = build_nc(cap)
    return _CACHED[cap]


def _host_prep(inputs, embeddings):
    """Sort points into per-core, per-grid-row chunks; build W matrices."""
    inputs = np.ascontiguousarray(np.asarray(inputs), dtype=np.float32)
    embeddings = np.asarray(embeddings)
    x = inputs * np.float32(RES - 1)
    xi = np.floor(x).astype(np.int32)
    xf = x - np.floor(x)
    xi0, xi1 = xi[:, 0], xi[:, 1]
    xf0, xf1 = xf[:, 0].astype(np.float32), xf[:, 1].astype(np.float32)
    wa = (1 - xf0) * (1 - xf1)  # row r    , col xi1
    wb = (1 - xf0) * xf1        # row r    , col xi1+1
    wc = xf0 * (1 - xf1)        # row r+128, col xi1
    wd = xf0 * xf1              # row r+128, col xi1+1

    core = xi0 // G_PER_CORE
    g_local = xi0 - core * G_PER_CORE
    n_per_row = np.bincount(xi0, minlength=RES)
    cap = max(DEF_CAP, int(np.ceil(n_per_row.max() / P)))
    nch = G_PER_CORE * cap
    cap_pts = cap * P

    # stable sort by (core, g_local) then assign slot within grid row
    order = np.argsort(core * G_PER_CORE + g_local, kind="stable")
    sorted_row = xi0[order]
    # slot index within each grid row (0..n_row-1), rows in sorted order
    row_starts = np.zeros(RES + 1, dtype=np.int64)
    np.cumsum(n_per_row, out=row_starts[1:])
    slot_in_row = np.arange(B_TOTAL, dtype=np.int64) - row_starts[sorted_row]

    # destination slot: (core, g_local*cap_pts + slot)
    dst_core = sorted_row // G_PER_CORE
    dst_slot = (sorted_row % G_PER_CORE) * cap_pts + slot_in_row
    assert slot_in_row.max() < cap_pts

    # Build W: [n_cores, 128 (k), nch, 2, 128 (m)] fp16
    W = np.zeros((N_CORES, P, nch, 2, P), dtype=np.float16)
    ch = dst_slot // P
    m = dst_slot % P
    c1 = xi1[order]
    W[dst_core, c1, ch, 0, m] = wa[order]
    W[dst_core, c1 + 1, ch, 0, m] = wb[order]
    W[dst_core, c1, ch, 1, m] = wc[order]
    W[dst_core, c1 + 1, ch, 1, m] = wd[order]

    # original global index of the point in each (core, slot); -1 = dummy
    orig = np.full((N_CORES, nch * P), -1, dtype=np.int64)
    orig[dst_core, dst_slot] = order

    # Band tables, fp16: core c gets table rows [16c*128, (16c+17)*128)
    emb16 = embeddings.astype(np.float16)
    bands = []
    for c in range(N_CORES):
        lo = c * G_PER_CORE * P
        hi = min(lo + BAND_ROWS * P, RES * P)
        band = np.zeros((BAND_ROWS * P, D), dtype=np.float16)
        band[: hi - lo] = emb16[lo:hi]
        bands.append(band.reshape(BAND_ROWS, P, D))

    in_maps = [
        {"w": np.ascontiguousarray(W[c].reshape(P, nch * 2 * P)), "band": bands[c]}
        for c in range(N_CORES)
    ]
    return {"in_maps": in_maps, "orig": orig, "cap": cap, "nch": nch}


def make_core_inputs(inputs: np.ndarray, embeddings: np.ndarray) -> list:
    global _LAST_PREP
    _LAST_PREP = _host_prep(inputs, embeddings)
    return _LAST_PREP["in_maps"]


def _out_rows(out_core: np.ndarray, nch: int) -> np.ndarray:
    """Device out fp16 -> [nch*128, 1024] rows."""
    return out_core.reshape(nch * P, D)


def core_output_global(out_core: np.ndarray, core: int):
    """Map one core's raw device output to (global_indices, fp32 values)."""
    prep = _LAST_PREP
    rows = _out_rows(out_core, prep["nch"])
    orig = prep["orig"][core]
    mask = orig >= 0
    return orig[mask], rows[mask].astype(np.float32)


def kernel(inputs: np.ndarray, embeddings: np.ndarray) -> np.ndarray:
    from concourse.bass_utils import run_bass_kernel_spmd

    in_maps = make_core_inputs(inputs, embeddings)
    prep = _LAST_PREP
    nc = _get_nc(prep["cap"])
    res = run_bass_kernel_spmd(nc, in_maps, core_ids=list(range(N_CORES)))
    out = np.empty((B_TOTAL, D), dtype=np.float32)
    covered = 0
    for c in range(N_CORES):
        gidx, vals = core_output_global(res.results[c]["out"], c)
        out[gidx] = vals
        covered += len(gidx)
    assert covered == B_TOTAL, f"only {covered} of {B_TOTAL} points covered"
    return out


if __name__ == "__main__":
    nc = build_nc()
    print("built ok")
